# revision 1
# baseline (speedup 1.0000x reference)
"""Cascaded-attention GRU recurrence on 8 NeuronCores (Bass/Tile).

Problem: B=128, T=75, D=512, V=28. Data-parallel over batch: 16 batch rows
per core, weights replicated. Per-core recurrence over 75 steps with two
independent batch half-groups (8 rows each) pipelined against each other.

Key layout choices (per core, BL=16 local batch):
- d-on-partition layout for the big tensors: UaH' = x@Ua + Ba1 + Ba2 stored
  [128(d-chunk), 4(chunk), 16(b), 75(t)]; per-step tanh runs as one ACT
  instruction per half-group.
- WaS bias (state@Wa, changes per step) broadcast over t via a stride-0 AP
  on the DVE tensor_tensor add.
- scores contracted against Va via PE with a column-replicated Va (VaSEL) so
  the result lands partition-major-ish as REP[75, (b,t)] (all rows equal);
  the free->partition transpose of the softmax weights is done by masking
  REP with a constant diagonal and free-reducing (DVE), yielding
  scoresT[75(t), b].
- softmax normalization deferred: unnormalized exp(scoresT) drives
  block-diag matmuls against XKC = x@[gru_kernel|Co] (precomputed on
  device), producing xm/CoC directly ([8, 112] per group); 1/Z folded into
  the GRU gate/output scalar ops.
- sigmoid via tanh (same ACT table set as exp): sigmoid(a) = .5 + .5*tanh(a/2),
  with gru_rec_kernel pre-halved on host so gate args come out right.
- embedding lookup: softmax probs cast to int32 are 0 unless pred == 1.0, so
  emb[idx]@Wo == w0 + (w1-w0)*[pred >= 1], w = emb@Wo (exact).
- Ba3 dropped (softmax shift invariance). gru_bias[1] folded into the hm
  matmul; gru_bias[0] assumed zero (asserted) as in the problem setup.
"""

import numpy as np

B, T, D, V = 128, 75, 512, 28
NCORES = 8
BL = B // NCORES        # 16 batch rows per core
G = 2                   # half-groups per core
BG = BL // G            # 8 rows per group
SUBS = [(0, 6), (6, 6), (12, 4)]  # softmax sub-groups (psum bank = 512 f32)
NC_, CH = 128, D // 128  # partitions, d-chunks
USE_GP_BIAS = False
USE_GP_SMB = False
USE_TMODE = False  # is_transpose matmuls: 4x faster fp32 weight path


def _build(nc, tc, tile, bass, mybir, gru_b0_nonzero, steps=T):
    f32 = mybir.dt.float32
    Act = mybir.ActivationFunctionType
    Op = mybir.AluOpType

    # ---------------- DRAM I/O ----------------
    dr = {}
    def din(name, shape):
        dr[name] = nc.dram_tensor(name, shape, f32, kind="ExternalInput")
        return dr[name]

    x_dmaj = din("x_dmaj", [NC_, CH, BL, T])
    ua_k = din("ua_k", [NC_, CH, CH, 128])
    ba12 = din("ba12", [NC_, CH])
    wa = din("wa", [V, D])
    vasel = din("vasel", [NC_, CH, T])
    w2 = din("w2", [NC_, CH, 112])
    wrec_h = din("wrec_h", [V + 1, 84])    # [0.5*gru_rec_kernel; 0.5*gru_bias1]
    uo = din("uo", [V + 1, V])             # [Uo; Bo + w0]
    diag = din("diag", [T, T])
    i16 = din("i16", [BL, BL])
    onesrow = din("onesrow", [1, BL])
    twos75 = din("twos75", [T, 1])
    dwrep = din("dwrep", [BG, V])          # w1-w0, replicated
    if gru_b0_nonzero:
        b0rep = din("b0rep", [BG, 84])
    y_out = nc.dram_tensor("y", [BL, T, V], f32, kind="ExternalOutput")

    import contextlib
    ctx = contextlib.ExitStack()
    with ctx:
        cst = ctx.enter_context(tc.tile_pool(name="cst", bufs=1))
        wrk = ctx.enter_context(tc.tile_pool(name="wrk", bufs=2))
        wrk3 = ctx.enter_context(tc.tile_pool(name="wrk3", bufs=3))
        pwast = ctx.enter_context(tc.tile_pool(name="pwast", bufs=1, space="PSUM"))
        pbt = ctx.enter_context(tc.tile_pool(name="pbt", bufs=2, space="PSUM"))
        prep = ctx.enter_context(tc.tile_pool(name="prep", bufs=3, space="PSUM"))
        pxm = ctx.enter_context(tc.tile_pool(name="pxm", bufs=2, space="PSUM"))

        # ---------------- constants into SBUF ----------------
        t_x = cst.tile([NC_, CH, BL, T], f32, tag="t_x")
        t_ua = cst.tile([NC_, CH, CH, 128], f32, tag="t_ua")
        t_ba12 = cst.tile([NC_, CH], f32, tag="t_ba12")
        t_wa = cst.tile([V, D], f32, tag="t_wa")
        t_vas = cst.tile([NC_, CH, T], f32, tag="t_vas")
        t_w2 = cst.tile([NC_, CH, 112], f32, tag="t_w2")
        t_wrec = cst.tile([V + 1, 84], f32, tag="t_wrec")
        t_uo = cst.tile([V + 1, V], f32, tag="t_uo")
        t_diag = cst.tile([T, T], f32, tag="t_diag")
        t_i16 = cst.tile([BL, BL], f32, tag="t_i16")
        t_two = cst.tile([T, 1], f32, tag="t_two")
        nc.sync.dma_start(t_two[:], twos75[:])
        t_dw = cst.tile([BG, V], f32, tag="t_dw")
        for tt, d_ in [(t_x, x_dmaj), (t_ua, ua_k), (t_ba12, ba12), (t_wa, wa),
                       (t_vas, vasel), (t_w2, w2), (t_wrec, wrec_h),
                       (t_uo, uo), (t_diag, diag),
                       (t_i16, i16), (t_dw, dwrep)]:
            nc.sync.dma_start(tt[:], d_[:])
        if gru_b0_nonzero:
            t_b0 = cst.tile([BG, 84], f32, tag="t_b0")
            nc.sync.dma_start(t_b0[:], b0rep[:])

        # persistent state/work tiles
        t_uahp = cst.tile([NC_, CH, BL, T], f32, tag="t_uahp")     # x@Ua + Ba1 + Ba2
        t_xkc = cst.tile([T, BL, 113], f32, tag="t_xkc")           # x@[gruK|Co|1]
        t_smb = cst.tile([T, BL * BL], f32, tag="t_smb")           # block-diag exp(scoresT)
        t_out = []
        for g in range(G):
            t_out_g = cst.tile([BG, T, V], f32, tag=f"t_out{g}")
            if steps < T:  # truncated builds (timing/sim only): avoid
                nc.vector.memset(t_out_g[:], 0.0)  # uninit reads at final DMA
            t_out.append(t_out_g)
        nc.vector.memset(t_smb[:], 0.0)

        # ---------------- preamble: UaH' = x@Ua + (Ba1+Ba2) ----------------
        NSL, SL = 3, 400  # bt slices per e-chunk
        for ec in range(CH):
            uah_flat = t_uahp[:, ec, :, :].rearrange("p b t -> p (b t)")
            for i in range(NSL):
                ps = prep.tile([NC_, SL], f32, tag="prep")
                for dc in range(CH):
                    x_sl = t_x[:, dc, :, :].rearrange("p b t -> p (b t)")[
                        :, i * SL:(i + 1) * SL]
                    nc.tensor.matmul(ps[:], t_ua[:, dc, ec, :], x_sl,
                                     start=(dc == 0), stop=(dc == CH - 1))
                nc.scalar.activation(uah_flat[:, i * SL:(i + 1) * SL], ps[:],
                                     Act.Identity, bias=t_ba12[:, ec:ec + 1],
                                     scale=1.0)

        # ---------------- preamble: XKC = x@[gruK|Co], ones col ----------------
        for b in range(BL):
            ps = pxm.tile([T, 112], f32, tag="pxm")
            for dc in range(CH):
                nc.tensor.matmul(ps[:], t_x[:, dc, b, :], t_w2[:, dc, :],
                                 start=(dc == 0), stop=(dc == CH - 1))
            nc.vector.tensor_copy(t_xkc[:, b, 0:112], ps[:])
        ones_col = bass.AP(t_xkc.tensor, t_xkc[:].offset + 112,
                           [list(t_xkc[:].ap[0]), [113, BL]])
        nc.vector.memset(ones_col, 1.0)

        # ---------------- recurrent state ----------------
        state = []   # [8,28] b-major per group
        for g in range(G):
            sg = wrk.tile([BG, V], f32, tag=f"state{g}")
            nc.vector.memset(sg[:], 0.0)
            state.append(sg)
        stateT = wrk.tile([V + 1, BL], f32, tag="stateT")  # shared; row V = ones
        nc.vector.memset(stateT[0:V, :], 0.0)
        nc.sync.dma_start(stateT[V:V + 1, :], onesrow[:])

        # ---------------- the 75 steps ----------------
        has_gp_tt = hasattr(nc.gpsimd, "tensor_tensor")
        for s in range(steps):
            new_state = [None] * G
            bt_list = [None] * G
            gorder = (0, 1)
            tanhY = wrk.tile([NC_, CH, BL, T], f32, tag="tanhY")
            # --- phase 0 (shared): WaS^T for both groups ---
            wast_ps = pwast.tile([NC_, 72], f32, tag="wastps")
            if s > 0:
                for c in range(CH):
                    nc.tensor.matmul(wast_ps[:, c * 16:(c + 1) * 16],
                                     t_wa[:, c * 128:(c + 1) * 128],
                                     stateT[0:V, :], start=True, stop=True,
                                     is_transpose=USE_TMODE)
                if has_gp_tt and USE_GP_BIAS:
                    # SBUF copy of the c2/c3 half for GPSIMD (no PSUM there)
                    wast_sb = wrk.tile([NC_, 32], f32, tag="wastsb")
                    nc.vector.tensor_copy(wast_sb[:], wast_ps[:, 32:64])
            # --- phase 1 (per group): state matmuls, bias-add, tanh ---
            for g in gorder:
                sg = state[g]
                sTg = stateT[:, g * BG:(g + 1) * BG]  # [29, 8]; row 28 = 1
                bs = g * BG
                bt_ps = pbt.tile([BG, 256], f32, tag="btps")
                bt_list[g] = bt_ps
                nc.tensor.matmul(bt_ps[:, 0:84], sTg, t_wrec[:],
                                 start=True, stop=True)
                nc.tensor.matmul(bt_ps[:, 84:112], sTg, t_uo[:],
                                 start=True, stop=True)

                ty_out = tanhY[:, :, bs:bs + BG, :]
                if s > 0:
                    Yg = wrk.tile([NC_, CH, BG, T], f32, tag=f"Y{g}")

                    def bias_add(eng, src, off, c0, cn):
                        # WaS^T broadcast over t (src: PSUM for DVE, SBUF copy
                        # for GPSIMD which cannot access PSUM)
                        w_sl = src[:, off + bs:off + bs + 8]
                        w_bc = bass.AP(
                            w_sl.tensor, w_sl.offset,
                            [list(w_sl.ap[0]), [16, cn], [1, 8], [0, T]])
                        eng.tensor_tensor(Yg[:, c0:c0 + cn, :, :],
                                          t_uahp[:, c0:c0 + cn, bs:bs + BG, :],
                                          w_bc, Op.add)
                    # split chunk-pairs so tanh(c01) overlaps bias-add(c23);
                    # c23 on GPSIMD runs concurrently with DVE's c01
                    if has_gp_tt and USE_GP_BIAS:
                        bias_add(nc.gpsimd, wast_sb, 0, 2, 2)
                        bias_add(nc.vector, wast_ps, 0, 0, 2)
                    else:
                        bias_add(nc.vector, wast_ps, 0, 0, 2)
                        bias_add(nc.vector, wast_ps, 32, 2, 2)
                    nc.scalar.activation(tanhY[:, 0:2, bs:bs + BG, :],
                                         Yg[:, 0:2, :, :], Act.Tanh)
                    nc.scalar.activation(tanhY[:, 2:4, bs:bs + BG, :],
                                         Yg[:, 2:4, :, :], Act.Tanh)
                else:
                    nc.scalar.activation(ty_out, t_uahp[:, :, bs:bs + BG, :],
                                         Act.Tanh)

            # --- phase 2 (shared, sub-granular): scoresT -> exp -> SmBlk ->
            # xm/CoC chunks emitted per sub-group so group tails start early
            scT = wrk.tile([T, BL], f32, tag="scT")
            expT = wrk.tile([T, BL], f32, tag="expT")
            xm_list = [pxm.tile([BG, 113], f32, tag="pxm", name=f"xm{g}_{s}")
                       for g in range(G)]
            for b0, nb in SUBS:
                rep_ps = prep.tile([T, 6 * T], f32, tag="prep")
                rep = rep_ps[:].rearrange("p (b t) -> p b t", b=6)[:, 0:nb, :]
                for c in range(CH):
                    nc.tensor.matmul(rep, t_vas[:, c, :],
                                     tanhY[:, c, b0:b0 + nb, :],
                                     start=(c == 0), stop=(c == CH - 1),
                                     is_transpose=USE_TMODE)
                msk = wrk3.tile([T, 6, T], f32, tag="msk")
                d_ap = t_diag[:]
                d_bc = bass.AP(d_ap.tensor, d_ap.offset,
                               [list(d_ap.ap[0]), [0, nb], list(d_ap.ap[1])])
                nc.vector.tensor_tensor(msk[:, 0:nb, :], rep, d_bc, Op.mult)
                nc.vector.tensor_reduce(scT[:, b0:b0 + nb], msk[:, 0:nb, :],
                                        mybir.AxisListType.X, Op.add)
                nc.scalar.activation(expT[:, b0:b0 + nb], scT[:, b0:b0 + nb],
                                     Act.Exp)
                smb_dst = bass.AP(t_smb.tensor, t_smb[:].offset + 17 * b0,
                                  [list(t_smb[:].ap[0]), [17, nb]])
                nc.vector.tensor_copy(smb_dst, expT[:, b0:b0 + nb])
                for b in range(b0, b0 + nb):
                    g = b // BG
                    bs = g * BG
                    nc.tensor.matmul(
                        xm_list[g][:],
                        t_smb[:, 16 * b + bs:16 * b + bs + BG],
                        t_xkc[:, b, :],
                        start=(b == bs), stop=(b == bs + BG - 1))
                for g in range(G):  # groups whose expT slice just completed
                    if b0 < (g + 1) * BG <= b0 + nb:
                        nc.tensor.matmul(bt_list[g][:, 112:113],
                                         expT[:, g * BG:(g + 1) * BG],
                                         t_two[:], start=True, stop=True)

            # --- phase 3 (per group): recip, gates, state, pred ---
            for g in gorder:
                sg = state[g]
                bs = g * BG
                bt_ps = bt_list[g]
                xm_ps = xm_list[g]
                hm_sb = wrk.tile([BG, 84], f32, tag=f"hm{g}")
                nc.vector.tensor_copy(hm_sb[:], bt_ps[:, 0:84])
                rhalf = wrk.tile([BG, 1], f32, tag=f"rhalf{g}")
                nc.vector.reciprocal(rhalf[:], bt_ps[:, 112:113])
                rfull = wrk.tile([BG, 1], f32, tag=f"rfull{g}")
                nc.vector.tensor_scalar(rfull[:], rhalf[:], 2.0, None, Op.mult)

                # --- GRU gates ---
                zr = wrk.tile([BG, 56], f32, tag=f"zr{g}")
                nc.vector.scalar_tensor_tensor(zr[:], xm_ps[:, 0:56], rhalf[:],
                                               hm_sb[:, 0:56], Op.mult, Op.add)
                if gru_b0_nonzero:
                    nc.vector.tensor_tensor(zr[:], zr[:], t_b0[:, 0:56], Op.add)
                tz = wrk.tile([BG, 56], f32, tag=f"tz{g}")
                nc.scalar.activation(tz[:], zr[:], Act.Tanh)
                s1 = wrk.tile([BG, V], f32, tag=f"s1{g}")
                nc.vector.scalar_tensor_tensor(s1[:], tz[:, V:56], 1.0,
                                               hm_sb[:, 56:84], Op.add, Op.mult)
                ah = wrk.tile([BG, V], f32, tag=f"ah{g}")
                nc.vector.scalar_tensor_tensor(ah[:], xm_ps[:, 56:84], rfull[:],
                                               s1[:], Op.mult, Op.add)
                if gru_b0_nonzero:
                    nc.vector.tensor_tensor(ah[:], ah[:], t_b0[:, 56:84], Op.add)
                hh = wrk.tile([BG, V], f32, tag=f"hh{g}")
                nc.scalar.activation(hh[:], ah[:], Act.Tanh)
                d1 = wrk.tile([BG, V], f32, tag=f"d1{g}")
                nc.vector.tensor_sub(d1[:], sg[:], hh[:])
                d2 = wrk.tile([BG, V], f32, tag=f"d2{g}")
                nc.vector.tensor_add(d2[:], sg[:], hh[:])
                m1 = wrk.tile([BG, V], f32, tag=f"m1{g}")
                nc.vector.tensor_mul(m1[:], tz[:, 0:V], d1[:])
                ns = wrk.tile([BG, V], f32, tag=f"state{g}")
                nc.vector.tensor_tensor(ns[:], m1[:], d2[:], Op.add)
                nc.vector.tensor_scalar(ns[:], ns[:], 0.5, None, Op.mult)

                # --- stateT for next step (into shared stateT col-slice) ---
                nc.tensor.transpose(wast_ps[0:V, 64:72], ns[:], t_i16[0:BG, 0:BG])
                nc.vector.tensor_copy(stateT[0:V, bs:bs + BG],
                                      wast_ps[0:V, 64:72])

                # --- pred logits + softmax (Bo+w0 folded into UoH psum) ---
                l1 = wrk.tile([BG, V], f32, tag=f"l1{g}")
                if s > 0:
                    l2 = wrk.tile([BG, V], f32, tag=f"l2{g}")
                    nc.vector.scalar_tensor_tensor(l2[:], t_out[g][:, s - 1, :],
                                                   1.0, t_dw[:], Op.is_ge,
                                                   Op.mult)
                    nc.vector.scalar_tensor_tensor(
                        l1[:], xm_ps[:, 84:112], rfull[:], l2[:],
                        Op.mult, Op.add)
                else:
                    nc.vector.tensor_scalar(l1[:], xm_ps[:, 84:112], rfull[:],
                                            None, Op.mult)
                logits = wrk.tile([BG, V], f32, tag=f"logits{g}")
                nc.vector.tensor_tensor(logits[:], l1[:], bt_ps[:, 84:112],
                                        Op.add)
                expP = wrk.tile([BG, V], f32, tag=f"expP{g}")
                zp = wrk.tile([BG, 1], f32, tag=f"zp{g}")
                nc.scalar.activation(expP[:], logits[:], Act.Exp,
                                     accum_out=zp[:])
                rp = wrk.tile([BG, 1], f32, tag=f"rp{g}")
                nc.vector.reciprocal(rp[:], zp[:])
                nc.vector.tensor_scalar(t_out[g][:, s, :], expP[:], rp[:],
                                        None, Op.mult)

                new_state[g] = ns

            state = new_state

        for g in range(G):
            nc.sync.dma_start(y_out[g * BG:(g + 1) * BG, :, :], t_out[g][:])
    return dr, y_out


_CACHE = {}


def _get_program(gru_b0_nonzero, steps=T):
    key = (bool(gru_b0_nonzero), steps)
    if key in _CACHE:
        return _CACHE[key]
    import concourse.bass as bass
    import concourse.bacc as bacc
    import concourse.tile as tile
    from concourse import mybir

    nc = bacc.Bacc("TRN2", target_bir_lowering=False, debug=False,
                   num_devices=NCORES)
    with tile.TileContext(nc) as tc:
        _build(nc, tc, tile, bass, mybir, gru_b0_nonzero, steps)
    nc.compile()
    _CACHE[key] = nc
    return nc


def _prep_core_inputs(inputs, core):
    x = inputs["x"]
    xs = np.ascontiguousarray(x[core * BL:(core + 1) * BL]).astype(np.float32)
    # [16,75,512] -> [128, chunk, b, t]
    x_dmaj = np.ascontiguousarray(
        xs.reshape(BL, T, CH, 128).transpose(3, 2, 0, 1))
    return x_dmaj


def _prep_weights(inputs):
    f = np.float32
    Ua = inputs["Ua"].astype(f)
    ua_k = np.ascontiguousarray(
        Ua.reshape(CH, 128, CH, 128).transpose(1, 0, 2, 3))
    ba = (inputs["Ba1"] + inputs["Ba2"]).astype(f).reshape(CH, 128)
    ba12 = np.ascontiguousarray(ba.T)
    Va = inputs["Va"].astype(f).reshape(CH, 128)
    vasel = np.ascontiguousarray(
        np.repeat(Va.T[:, :, None], T, axis=2))
    w2 = np.concatenate([inputs["gru_kernel"], inputs["Co"]], axis=1).astype(f)
    w2 = np.ascontiguousarray(w2.reshape(CH, 128, 112).transpose(1, 0, 2))
    w = (inputs["emb"].astype(f) @ inputs["Wo"].astype(f)).reshape(-1)
    w0, w1 = float(w[0]), float(w[1])
    gb = inputs["gru_bias"].astype(f)
    out = {
        "ua_k": ua_k, "ba12": ba12, "wa": inputs["Wa"].astype(f),
        "vasel": vasel, "w2": w2,
        "wrec_h": np.concatenate(
            [0.5 * inputs["gru_rec_kernel"].astype(f), 0.5 * gb[1:2]], axis=0),
        "uo": np.concatenate(
            [inputs["Uo"].astype(f), inputs["Bo"].astype(f) + w0], axis=0),
        "diag": np.eye(T, dtype=f),
        "i16": np.eye(BL, dtype=f),
        "onesrow": np.ones([1, BL], dtype=f),
        "twos75": np.full([T, 1], 2.0, dtype=f),
        "dwrep": np.full([BG, V], w1 - w0, dtype=f),
    }
    b0 = gb[0]
    if np.any(b0 != 0):
        out["b0rep"] = np.repeat(b0[None, :], BG, axis=0)
    return out, bool(np.any(b0 != 0))


def kernel(**inputs):
    from concourse.bass_utils import run_bass_kernel_spmd

    weights, b0nz = _prep_weights(inputs)
    nc = _get_program(b0nz)
    in_maps = []
    for core in range(NCORES):
        m = dict(weights)
        m["x_dmaj"] = _prep_core_inputs(inputs, core)
        in_maps.append(m)
    res = run_bass_kernel_spmd(nc, in_maps, core_ids=list(range(NCORES)))
    out = np.concatenate([res.results[c]["y"] for c in range(NCORES)], axis=0)
    return out.astype(np.float32)





# revision 3
# speedup vs baseline: 4.6634x; 4.6634x over previous
"""Cascaded-attention GRU recurrence on 8 NeuronCores (Bass/Tile), v2.

Problem: B=128, T=75, D=512, V=28. Data-parallel over batch: 16 batch rows
per core, weights replicated. Per-core recurrence over 75 steps.

Key layout/dtype choices (per core, BL=16 local batch):
- t-major fp16 layout for the big step tensors: UaH' = x@Ua + Ba1 + Ba2
  stored [128(d-chunk), 4(chunk), 75(t), 16(b)] fp16. The per-step bias add
  Yg = UaH' + WaS^T runs on DVE in 2x_1p mode (fp16, b innermost stride-1,
  WaS broadcast over t via a stride-0 middle AP dim) -- 2x faster than the
  fp32 b-major baseline. tanh(Yg) on ACT writes tanhY fp16.
- WaS^T = Wa^T @ stateT per step on PE (4 chunk matmuls into PSUM), copied
  once to an fp16 SBUF tile (ACT) for the DVE 2x bias add.
- scores: rep[t', (t,b)] = VaSEL^T @ tanhY per b-subgroup (psum bank = 512
  f32 => subgroups of 6/6/4 b). fp16 moving data = 1 cyc/row on PE (4x
  faster than the fp32 baseline). Diagonal extraction via constant-diag mask
  (DVE) + free-axis reduce -> scoresT[t, b] fp32.
- softmax normalization deferred: unnormalized exp(scoresT) (fp16) drives
  block-diag matmuls against XKC = x@[gru_kernel|Co] (fp16, precomputed on
  device) producing xm/CoC for all 16 rows in one PSUM tile; 1/Z folded
  into the GRU gate/output scalar ops.
- phase 3 (gates/pred) runs group-merged ([16, .] tiles) to halve the
  small-op count; PSUM->SBUF copies (hm, stateT) moved to ACT.
- sigmoid via tanh (same ACT table set as exp): sigmoid(a) = .5 + .5*tanh(a/2),
  with gru_rec_kernel pre-halved on host so gate args come out right.
- embedding lookup: softmax probs cast to int32 are 0 unless pred == 1.0, so
  emb[idx]@Wo == w0 + (w1-w0)*[pred >= 1], w = emb@Wo (exact).
- Ba3 dropped (softmax shift invariance). gru_bias[1] folded into the hm
  matmul; gru_bias[0] handled if nonzero.
"""

import numpy as np

B, T, D, V = 128, 75, 512, 28
NCORES = 8
BL = B // NCORES        # 16 batch rows per core
G = 2                   # half-groups per core (phases 0-2 pipelining)
BG = BL // G            # 8 rows per group
SUBS = [(0, 6), (6, 6), (12, 4)]  # softmax sub-groups (psum bank = 512 f32)
NC_, CH = 128, D // 128  # partitions, d-chunks
USE_POOL_REDUCE = False  # scoresT reduce on GPSIMD/Pool engine
USE_POOL_P3 = False      # a few phase-3 elementwise ops on GPSIMD/Pool


def _build(nc, tc, tile, bass, mybir, gru_b0_nonzero, steps=T):
    f32 = mybir.dt.float32
    f16 = mybir.dt.float16
    Act = mybir.ActivationFunctionType
    Op = mybir.AluOpType

    # ---------------- DRAM I/O ----------------
    dr = {}
    def din(name, shape, dt=f32):
        dr[name] = nc.dram_tensor(name, shape, dt, kind="ExternalInput")
        return dr[name]

    x_dmaj = din("x_dmaj", [NC_, CH, T, BL])          # t-major!
    ua_k = din("ua_k", [NC_, CH, CH, 128])
    ba12 = din("ba12", [NC_, CH])
    wa = din("wa", [V, D])
    vasel = din("vasel", [NC_, CH, T], f16)
    w2 = din("w2", [NC_, CH, 112])
    wrec_h = din("wrec_h", [V + 1, 84])    # [0.5*gru_rec_kernel; 0.5*gru_bias1]
    uo = din("uo", [V + 1, V])             # [Uo; Bo + w0]
    diag = din("diag", [T, T])
    i16 = din("i16", [BL, BL])
    onesrow = din("onesrow", [1, BL])
    twos75 = din("twos75", [T, 1], f16)
    dwrep = din("dwrep", [BL, V])          # w1-w0, replicated
    if gru_b0_nonzero:
        b0rep = din("b0rep", [BL, 84])
    y_out = nc.dram_tensor("y", [BL, T, V], f32, kind="ExternalOutput")

    import contextlib
    ctx = contextlib.ExitStack()
    with ctx:
        cst = ctx.enter_context(tc.tile_pool(name="cst", bufs=1))
        wrk = ctx.enter_context(tc.tile_pool(name="wrk", bufs=2))
        wrk3 = ctx.enter_context(tc.tile_pool(name="wrk3", bufs=3))
        pwast = ctx.enter_context(tc.tile_pool(name="pwast", bufs=1, space="PSUM"))
        pbt = ctx.enter_context(tc.tile_pool(name="pbt", bufs=2, space="PSUM"))
        prep = ctx.enter_context(tc.tile_pool(name="prep", bufs=3, space="PSUM"))
        pxm = ctx.enter_context(tc.tile_pool(name="pxm", bufs=2, space="PSUM"))

        # ---------------- constants into SBUF ----------------
        t_x = cst.tile([NC_, CH, T, BL], f32, tag="t_x")
        t_ua = cst.tile([NC_, CH, CH, 128], f32, tag="t_ua")
        t_ba12 = cst.tile([NC_, CH], f32, tag="t_ba12")
        t_wa = cst.tile([V, D], f32, tag="t_wa")
        t_vas = cst.tile([NC_, CH, T], f16, tag="t_vas")
        t_w2 = cst.tile([NC_, CH, 112], f32, tag="t_w2")
        t_wrec = cst.tile([V + 1, 84], f32, tag="t_wrec")
        t_uo = cst.tile([V + 1, V], f32, tag="t_uo")
        t_diag = cst.tile([T, T], f32, tag="t_diag")
        t_i16 = cst.tile([BL, BL], f32, tag="t_i16")
        t_two = cst.tile([T, 1], f16, tag="t_two")
        nc.sync.dma_start(t_two[:], twos75[:])
        t_dw = cst.tile([BL, V], f32, tag="t_dw")
        for tt, d_ in [(t_x, x_dmaj), (t_ua, ua_k), (t_ba12, ba12), (t_wa, wa),
                       (t_vas, vasel), (t_w2, w2), (t_wrec, wrec_h),
                       (t_uo, uo), (t_diag, diag),
                       (t_i16, i16), (t_dw, dwrep)]:
            nc.sync.dma_start(tt[:], d_[:])
        if gru_b0_nonzero:
            t_b0 = cst.tile([BL, 84], f32, tag="t_b0")
            nc.sync.dma_start(t_b0[:], b0rep[:])

        # persistent state/work tiles
        t_uahp = cst.tile([NC_, CH, T, BL], f16, tag="t_uahp")  # x@Ua + Ba
        t_xkc = cst.tile([T, BL, 113], f16, tag="t_xkc")        # x@[gruK|Co|1]
        t_smb = cst.tile([T, BL * BL], f16, tag="t_smb")        # blkdiag expT
        t_out = cst.tile([BL, T, V], f32, tag="t_out")
        if steps < T:  # truncated builds (timing/sim only): avoid
            nc.vector.memset(t_out[:], 0.0)  # uninit reads at final DMA
        nc.vector.memset(t_smb[:], 0.0)

        # ---------------- preamble: UaH' = x@Ua + (Ba1+Ba2) ----------------
        NSL, SL = 3, 400  # (t,b) slices per e-chunk
        for ec in range(CH):
            uah_flat = t_uahp[:, ec, :, :].rearrange("p t b -> p (t b)")
            for i in range(NSL):
                ps = prep.tile([NC_, SL], f32, tag="prep")
                for dc in range(CH):
                    x_sl = t_x[:, dc, :, :].rearrange("p t b -> p (t b)")[
                        :, i * SL:(i + 1) * SL]
                    nc.tensor.matmul(ps[:], t_ua[:, dc, ec, :], x_sl,
                                     start=(dc == 0), stop=(dc == CH - 1))
                nc.scalar.activation(uah_flat[:, i * SL:(i + 1) * SL], ps[:],
                                     Act.Identity, bias=t_ba12[:, ec:ec + 1],
                                     scale=1.0)

        # ---------------- preamble: XKC = x@[gruK|Co], ones col ----------------
        for b in range(BL):
            ps = pxm.tile([T, 113], f32, tag="pxm")
            for dc in range(CH):
                nc.tensor.matmul(ps[:, 0:112], t_x[:, dc, :, b],
                                 t_w2[:, dc, :],
                                 start=(dc == 0), stop=(dc == CH - 1))
            nc.vector.tensor_copy(t_xkc[:, b, 0:112], ps[:, 0:112])
        ones_col = bass.AP(t_xkc.tensor, t_xkc[:].offset + 112,
                           [list(t_xkc[:].ap[0]), [113, BL]])
        nc.vector.memset(ones_col, 1.0)

        # ---------------- recurrent state ----------------
        sg = wrk.tile([BL, V], f32, tag="state")
        nc.vector.memset(sg[:], 0.0)
        stateT = wrk.tile([V + 1, BL], f32, tag="stateT")  # row V = ones
        nc.vector.memset(stateT[0:V, :], 0.0)
        nc.sync.dma_start(stateT[V:V + 1, :], onesrow[:])

        red_eng = nc.gpsimd if USE_POOL_REDUCE else nc.vector
        p3_eng = nc.gpsimd if USE_POOL_P3 else nc.vector

        # ---------------- the 75 steps ----------------
        for s in range(steps):
            tanhY = wrk.tile([NC_, CH, T, BL], f16, tag="tanhY")
            # --- phase 0 (shared): WaS^T for all 16 rows ---
            wast_ps = pwast.tile([NC_, 80], f32, tag="wastps")
            if s > 0:
                for c in range(CH):
                    nc.tensor.matmul(wast_ps[:, c * 16:(c + 1) * 16],
                                     t_wa[:, c * 128:(c + 1) * 128],
                                     stateT[0:V, :], start=True, stop=True)
                wsb = wrk.tile([NC_, 64], f16, tag="wsb")
                nc.scalar.activation(wsb[:], wast_ps[:, 0:64], Act.Identity)

            # --- phase 1 (per group): bias-add (DVE 2x fp16) + tanh ---
            for g in range(G):
                bs = g * BG
                if s > 0:
                    for cp in (0, 2):  # chunk pairs
                        Yg = wrk.tile([NC_, 2, T, BG], f16, tag="Yg")
                        w_sl = wsb[:]
                        w_bc = bass.AP(
                            w_sl.tensor, w_sl.offset + cp * 16 + bs,
                            [list(w_sl.ap[0]), [16, 2], [0, T], [1, BG]])
                        nc.vector.tensor_tensor(
                            Yg[:], t_uahp[:, cp:cp + 2, :, bs:bs + BG],
                            w_bc, Op.add)
                        nc.scalar.activation(
                            tanhY[:, cp:cp + 2, :, bs:bs + BG], Yg[:],
                            Act.Tanh)
                else:
                    nc.scalar.activation(tanhY[:, :, :, bs:bs + BG],
                                         t_uahp[:, :, :, bs:bs + BG],
                                         Act.Tanh)

            # --- phase 2 (shared, sub-granular): scoresT -> exp -> SmBlk ->
            # xm/CoC chunks emitted per sub-group so downstream starts early
            scT = wrk.tile([T, BL], f32, tag="scT")
            expT = wrk.tile([T, BL], f16, tag="expT")
            xm_ps = pxm.tile([BL, 113], f32, tag="pxm", name=f"xm_{s}")
            bt_ps = pbt.tile([BL, 256], f32, tag="btps")
            nc.tensor.matmul(bt_ps[:, 0:84], stateT[:], t_wrec[:],
                             start=True, stop=True)
            nc.tensor.matmul(bt_ps[:, 84:112], stateT[:], t_uo[:],
                             start=True, stop=True)
            for b0, nb in SUBS:
                rep_ps = prep.tile([T, 6 * T], f32, tag="prep")
                rep_fl = rep_ps[:, 0:nb * T]
                for c in range(CH):
                    nc.tensor.matmul(rep_fl, t_vas[:, c, :],
                                     tanhY[:, c, :, b0:b0 + nb],
                                     start=(c == 0), stop=(c == CH - 1))
                # rep columns are (t, b): view [t', t, b]
                rep3 = rep_ps[:, 0:nb * T].rearrange("p (t b) -> p t b", b=nb)
                msk = wrk3.tile([T, 6, T], f32, tag="msk")  # [t', b, t]
                d_ap = t_diag[:]
                d_bc = bass.AP(d_ap.tensor, d_ap.offset,
                               [list(d_ap.ap[0]), list(d_ap.ap[1]), [0, nb]])
                msk_w = bass.AP(msk.tensor, msk[:].offset,
                                [list(msk[:].ap[0]), [1, T], [T, nb]])
                nc.vector.tensor_tensor(msk_w, rep3, d_bc, Op.mult)
                red_eng.tensor_reduce(scT[:, b0:b0 + nb], msk[:, 0:nb, :],
                                      mybir.AxisListType.X, Op.add)
                nc.scalar.activation(expT[:, b0:b0 + nb], scT[:, b0:b0 + nb],
                                     Act.Exp)
                smb_dst = bass.AP(t_smb.tensor, t_smb[:].offset + 17 * b0,
                                  [list(t_smb[:].ap[0]), [17, nb]])
                nc.vector.tensor_copy(smb_dst, expT[:, b0:b0 + nb])
                for b in range(b0, b0 + nb):
                    nc.tensor.matmul(
                        xm_ps[:],
                        t_smb[:, 16 * b:16 * b + 16],
                        t_xkc[:, b, :],
                        start=(b == 0), stop=(b == BL - 1))
            nc.tensor.matmul(bt_ps[:, 112:113], expT[:], t_two[:],
                             start=True, stop=True)

            # --- phase 3 (merged): recip, gates, state, pred ---
            hm_sb = wrk.tile([BL, 84], f32, tag="hm")
            nc.scalar.activation(hm_sb[:], bt_ps[:, 0:84], Act.Identity)
            rhalf = wrk.tile([BL, 1], f32, tag="rhalf")
            nc.vector.reciprocal(rhalf[:], bt_ps[:, 112:113])
            rfull = wrk.tile([BL, 1], f32, tag="rfull")
            nc.vector.tensor_scalar(rfull[:], rhalf[:], 2.0, None, Op.mult)

            # --- GRU gates ---
            zr = wrk.tile([BL, 56], f32, tag="zr")
            nc.vector.scalar_tensor_tensor(zr[:], xm_ps[:, 0:56], rhalf[:],
                                           hm_sb[:, 0:56], Op.mult, Op.add)
            if gru_b0_nonzero:
                nc.vector.tensor_tensor(zr[:], zr[:], t_b0[:, 0:56], Op.add)
            tz = wrk.tile([BL, 56], f32, tag="tz")
            nc.scalar.activation(tz[:], zr[:], Act.Tanh)
            s1 = wrk.tile([BL, V], f32, tag="s1")
            nc.vector.scalar_tensor_tensor(s1[:], tz[:, V:56], 1.0,
                                           hm_sb[:, 56:84], Op.add, Op.mult)
            ah = wrk.tile([BL, V], f32, tag="ah")
            nc.vector.scalar_tensor_tensor(ah[:], xm_ps[:, 56:84], rfull[:],
                                           s1[:], Op.mult, Op.add)
            if gru_b0_nonzero:
                nc.vector.tensor_tensor(ah[:], ah[:], t_b0[:, 56:84], Op.add)
            hh = wrk.tile([BL, V], f32, tag="hh")
            nc.scalar.activation(hh[:], ah[:], Act.Tanh)
            d1 = wrk.tile([BL, V], f32, tag="d1")
            p3_eng.tensor_sub(d1[:], sg[:], hh[:])
            d2 = wrk.tile([BL, V], f32, tag="d2")
            p3_eng.tensor_add(d2[:], sg[:], hh[:])
            m1 = wrk.tile([BL, V], f32, tag="m1")
            nc.vector.tensor_mul(m1[:], tz[:, 0:V], d1[:])
            ns = wrk.tile([BL, V], f32, tag="state")
            nc.vector.tensor_tensor(ns[:], m1[:], d2[:], Op.add)
            nc.vector.tensor_scalar(ns[:], ns[:], 0.5, None, Op.mult)

            # --- stateT for next step ---
            nc.tensor.transpose(wast_ps[0:V, 64:80], ns[:], t_i16[:])
            nc.scalar.activation(stateT[0:V, :], wast_ps[0:V, 64:80],
                                 Act.Identity)

            # --- pred logits + softmax (Bo+w0 folded into UoH psum) ---
            l1 = wrk.tile([BL, V], f32, tag="l1")
            if s > 0:
                l2 = wrk.tile([BL, V], f32, tag="l2")
                p3_eng.scalar_tensor_tensor(l2[:], t_out[:, s - 1, :],
                                            1.0, t_dw[:], Op.is_ge,
                                            Op.mult)
                nc.vector.scalar_tensor_tensor(
                    l1[:], xm_ps[:, 84:112], rfull[:], l2[:],
                    Op.mult, Op.add)
            else:
                nc.vector.tensor_scalar(l1[:], xm_ps[:, 84:112], rfull[:],
                                        None, Op.mult)
            logits = wrk.tile([BL, V], f32, tag="logits")
            nc.vector.tensor_tensor(logits[:], l1[:], bt_ps[:, 84:112],
                                    Op.add)
            expP = wrk.tile([BL, V], f32, tag="expP")
            zp = wrk.tile([BL, 1], f32, tag="zp")
            nc.scalar.activation(expP[:], logits[:], Act.Exp,
                                 accum_out=zp[:])
            rp = wrk.tile([BL, 1], f32, tag="rp")
            nc.vector.reciprocal(rp[:], zp[:])
            p3_eng.tensor_scalar(t_out[:, s, :], expP[:], rp[:],
                                 None, Op.mult)
            sg = ns

        nc.sync.dma_start(y_out[:], t_out[:])
    return dr, y_out


_CACHE = {}


def _get_program(gru_b0_nonzero, steps=T):
    key = (bool(gru_b0_nonzero), steps)
    if key in _CACHE:
        return _CACHE[key]
    import concourse.bass as bass
    import concourse.bacc as bacc
    import concourse.tile as tile
    from concourse import mybir

    nc = bacc.Bacc("TRN2", target_bir_lowering=False, debug=False,
                   num_devices=NCORES)
    with tile.TileContext(nc) as tc:
        _build(nc, tc, tile, bass, mybir, gru_b0_nonzero, steps)
    nc.compile()
    _CACHE[key] = nc
    return nc


def _prep_core_inputs(inputs, core):
    x = inputs["x"]
    xs = np.ascontiguousarray(x[core * BL:(core + 1) * BL]).astype(np.float32)
    # [16,75,512] -> [128(d%128), chunk, t, b]
    x_dmaj = np.ascontiguousarray(
        xs.reshape(BL, T, CH, 128).transpose(3, 2, 1, 0))
    return x_dmaj


def _prep_weights(inputs):
    f = np.float32
    Ua = inputs["Ua"].astype(f)
    ua_k = np.ascontiguousarray(
        Ua.reshape(CH, 128, CH, 128).transpose(1, 0, 2, 3))
    ba = (inputs["Ba1"] + inputs["Ba2"]).astype(f).reshape(CH, 128)
    ba12 = np.ascontiguousarray(ba.T)
    Va = inputs["Va"].astype(f).reshape(CH, 128)
    vasel = np.ascontiguousarray(
        np.repeat(Va.T[:, :, None], T, axis=2)).astype(np.float16)
    w2 = np.concatenate([inputs["gru_kernel"], inputs["Co"]], axis=1).astype(f)
    w2 = np.ascontiguousarray(w2.reshape(CH, 128, 112).transpose(1, 0, 2))
    w = (inputs["emb"].astype(f) @ inputs["Wo"].astype(f)).reshape(-1)
    w0, w1 = float(w[0]), float(w[1])
    gb = inputs["gru_bias"].astype(f)
    out = {
        "ua_k": ua_k, "ba12": ba12, "wa": inputs["Wa"].astype(f),
        "vasel": vasel, "w2": w2,
        "wrec_h": np.concatenate(
            [0.5 * inputs["gru_rec_kernel"].astype(f), 0.5 * gb[1:2]], axis=0),
        "uo": np.concatenate(
            [inputs["Uo"].astype(f), inputs["Bo"].astype(f) + w0], axis=0),
        "diag": np.eye(T, dtype=f),
        "i16": np.eye(BL, dtype=f),
        "onesrow": np.ones([1, BL], dtype=f),
        "twos75": np.full([T, 1], 2.0, dtype=np.float16),
        "dwrep": np.full([BL, V], w1 - w0, dtype=f),
    }
    b0 = gb[0]
    if np.any(b0 != 0):
        out["b0rep"] = np.repeat(b0[None, :], BL, axis=0)
    return out, bool(np.any(b0 != 0))


def kernel(**inputs):
    from concourse.bass_utils import run_bass_kernel_spmd

    weights, b0nz = _prep_weights(inputs)
    nc = _get_program(b0nz)
    in_maps = []
    for core in range(NCORES):
        m = dict(weights)
        m["x_dmaj"] = _prep_core_inputs(inputs, core)
        in_maps.append(m)
    res = run_bass_kernel_spmd(nc, in_maps, core_ids=list(range(NCORES)))
    out = np.concatenate([res.results[c]["y"] for c in range(NCORES)], axis=0)
    return out.astype(np.float32)
